# revision 65
# baseline (speedup 1.0000x reference)
"""Trainium2 Bass kernel for nn_BasicBlock (binarized ResNet basic block).

Computation (see problem reference):
    residual = x
    out = psum_conv3x3(sign(x), sign(w1))        # 3x3 'same' conv, saturating acc
    out = bn1(out); out = hardtanh(out)
    out = psum_conv3x3(sign(out), sign(w2))
    out = bn2(out); out = out + residual; out = hardtanh(out)

Key facts exploited:
  * C=128 channels = one GROUP, 9 taps of |partial| <= 128 each, so the
    running accumulator magnitude is <= 9*128 = 1152 < THRESH=8000: the
    saturation clip NEVER binds. The conv is a plain 3x3 conv over sign
    values, all arithmetic exact small integers -> freely reorderable and
    exactly representable in fp8e4/bf16 inputs with fp32 PSUM accumulation.
  * sign(hardtanh(v)) == sign(v), so the first hardtanh can be folded into
    the sign feeding conv2.
  * Each conv = 9 shifted-window taps (K=C=128 on partitions) into one PSUM
    accumulation group over a zero-padded row-stride-64 fp8 sign image:
    4 fp8 DoubleRow matmuls (vertically adjacent tap pairs at +RW, plus the
    (r2,c0)+(r2,c1) pair via a col-shifted copy at +SHIFT) and 1 normal
    fp8 matmul for the odd tap (r2,c2).
  * Host-side prep: sign(x) is computed on the host and shipped as fp8
    (both the padded image and its col-shifted twin are plain DMAs from the
    same HBM tensor), and the residual arrives as x+b2 in bf16 (bias of the
    second BN folded in; sign() and +b2 are exact/cheap host prep like the
    weight binarization). The second conv's input sign image is produced
    on-chip by the Scalar engine (bn1+sign straight out of PSUM); its
    shifted twin is an SBUF->SBUF DMA on the otherwise idle DMA rings.
  * y is returned as bf16 (quantization ~2^-9 against a 2e-2 budget).

Sharding: data-parallel over batch: 64 images -> 8 cores x 8 images.
"""

import sys

sys.path.insert(0, "/opt/trn_rl_repo")

import numpy as np
import ml_dtypes

import concourse.bass as bass
import concourse.bacc as bacc
import concourse.mybir as mybir
import concourse.tile as tile
from concourse.bass_utils import run_bass_kernel_spmd

# ---------------------------------------------------------------- constants

N_CORES = 8
B, C, H, W = 64, 128, 56, 56
BL = B // N_CORES            # images per core
HP = H + 2                   # padded rows
RW = 64                      # padded row width (stride): 56 valid + pads,
                             # 64 so the DoubleRow plane step (+RW) is 16-aligned
CHUNK_ROWS = 8               # output rows per PSUM chunk
NFLAT = CHUNK_ROWS * RW      # 512 flat psum columns per chunk (one bank)
N_CHUNKS = H // CHUNK_ROWS   # 7
EPS = 1e-5
SHIFT = HP * RW              # offset of the col-shifted copy inside xs/ts
WCOLS = 2 * (4 * 256 + 128)  # fp8 weight table columns (2 convs x 1152)
PIECES = ((0, 11), (11, 24), (35, 23))  # sign-image DMA pieces (padded rows)

F32 = mybir.dt.float32
BF16 = mybir.dt.bfloat16
FP8 = mybir.dt.float8e4

# perf_mode for the odd (r2,c2) tap. DoublePixel compiles and produces
# correct results but is silently dropped before NEFF encoding (no perf_opt
# bit in the emitted instruction), so it buys nothing; keep None.
DP_SINGLE = None
# last image: alternate evictions between the DVE path and Scalar+DVE path
SPLIT_EVICT = False

_NC_CACHE = None


def _build_nc():
    """Build the per-core Bass module (same NEFF on all 8 cores)."""
    nc = bacc.Bacc("TRN2", debug=False)

    # host-binarized sign(x) in fp8, already laid out as the zero-padded
    # row-stride-64 image followed by its col-shifted twin (so every piece
    # DMA is contiguous on both sides), and the b2-biased residual in bf16
    s_d = nc.dram_tensor("s", [BL, C, 2 * SHIFT], FP8, kind="ExternalInput").ap()
    xr_d = nc.dram_tensor("xr", [BL, C, H, W], BF16, kind="ExternalInput").ap()
    # host-prepped fp8 weight tables, per conv: 3 DoubleRow pair tables
    # [cin, 2*cout] for (r0,r1) at c=0,1,2 then the (r2,c0)+(r2,c1) pair and
    # the plain (r2,c2) table
    w_d = nc.dram_tensor("w", [C, WCOLS], FP8, kind="ExternalInput").ap()
    # folded BN params per channel: [:,0]=inv1 [:,1]=b1 [:,2]=inv2
    bn_d = nc.dram_tensor("bn", [C, 4], F32, kind="ExternalInput").ap()
    y_d = nc.dram_tensor("y", [BL, C, H, W], BF16, kind="ExternalOutput").ap()

    SIGN = mybir.ActivationFunctionType.Sign
    DR = mybir.MatmulPerfMode.DoubleRow

    with tile.TileContext(nc) as tc:
        with (
            tc.tile_pool(name="sb", bufs=1) as sb,
            tc.tile_pool(name="psum", bufs=4, space="PSUM") as pspool,
        ):
            # -- startup: preload the SIGN activation table while DMAs run
            junk = sb.tile([C, 2], F32, name="junk")
            nc.vector.memset(junk[:], 0.0)
            nc.scalar.activation(junk[:, 1:2], junk[:, 0:1], SIGN)
            # fp8 scratch feeding the PE-clock warmup matmuls
            junk2 = sb.tile([C, 512], FP8, name="junk2")
            nc.vector.memset(junk2[:], 0.0)

            w_sb = sb.tile([C, WCOLS], FP8, name="wsb")
            bn_sb = sb.tile([C, 4], F32, name="bnsb")
            xs_t = [sb.tile([C, 2 * SHIFT], FP8, name=f"xs{j}") for j in range(3)]
            ts_t = [sb.tile([C, 2 * SHIFT], FP8, name=f"ts{j}") for j in range(3)]
            # ring depth 4: the write-after-read dependency of image i's
            # residual load then lands on image i-4's (long finished) final
            # eviction, so the issue never blocks its queue
            xr_t = [sb.tile([C, H, W], BF16, name=f"xr{j}") for j in range(4)]
            o_t = [sb.tile([C, H, W], BF16, name=f"o{j}") for j in range(2)]
            # f32 staging for the last image's Scalar+Pool eviction path
            tmp_t = [
                sb.tile([C, CHUNK_ROWS, W], F32, name=f"tmp{j}")
                for j in range(2)
            ]

            def xs3v(buf):
                return buf[:, 0:SHIFT].rearrange("p (h w) -> p h w", w=RW)

            def xsh3v(buf):
                return buf[:, SHIFT : 2 * SHIFT].rearrange(
                    "p (h w) -> p h w", w=RW
                )

            def zero_pads(eng, buf):
                b3 = xs3v(buf)
                eng.memset(b3[:, 0, :], 0.0)
                eng.memset(b3[:, HP - 1, :], 0.0)
                eng.memset(b3[:, 1 : HP - 1, 0:1], 0.0)
                eng.memset(b3[:, 1 : HP - 1, W + 1 : RW], 0.0)
                # last padded row of the shifted copy is all pad-derived
                eng.memset(buf[:, SHIFT + (HP - 1) * RW : 2 * SHIFT], 0.0)

            # ramp-ordered queues. Sync carries only the main sign-image
            # pieces (plus conv1 weights); gpsimd carries the shifted twins,
            # bn, conv2 weights and residuals — neither queue ever backs up
            # behind the other's latency-critical issues.
            nc.sync.dma_start(
                xs_t[0][:, 0 : 11 * RW], s_d[0, :, 0 : 11 * RW]
            )
            nc.sync.dma_start(w_sb[:, 0:1152], w_d[:, 0:1152])
            nc.sync.dma_start(
                xs_t[0][:, 11 * RW : 35 * RW], s_d[0, :, 11 * RW : 35 * RW]
            )
            nc.gpsimd.dma_start(
                xs_t[0][:, SHIFT : SHIFT + 11 * RW],
                s_d[0, :, SHIFT : SHIFT + 11 * RW],
            )
            nc.gpsimd.dma_start(bn_sb[:], bn_d[:])
            nc.gpsimd.dma_start(
                xs_t[0][:, SHIFT + 11 * RW : SHIFT + 35 * RW],
                s_d[0, :, SHIFT + 11 * RW : SHIFT + 35 * RW],
            )
            nc.gpsimd.dma_start(w_sb[:, 1152:WCOLS], w_d[:, 1152:WCOLS])

            def shift_dma(eng, buf, row0, nrows):
                """shifted[h, w] = main[h, w+1] for rows [row0, row0+nrows)
                via SBUF->SBUF DMA (pad cols supply the tail bytes)."""
                src = bass.AP(
                    tensor=buf.tensor,
                    offset=buf.offset + row0 * RW + 1,
                    ap=[buf.ap[0], [1, nrows * RW]],
                )
                dst = bass.AP(
                    tensor=buf.tensor,
                    offset=buf.offset + SHIFT + row0 * RW,
                    ap=[buf.ap[0], [1, nrows * RW]],
                )
                eng.dma_start(dst, src)

            def chunk_matmul(ps, src, conv_idx, h0, step, nrows=CHUNK_ROWS):
                """Emit matmul `step` (0..4) of one output chunk: 4 DoubleRow
                + 1 normal fp8 matmul.

                Steps 0..2 pair the vertically adjacent taps (r0,c)+(r1,c)
                (planes at +RW). Step 3 is the odd tap (r2,c2) as a normal
                matmul (before step 4 so a late shifted copy never stalls
                it); step 4 pairs (r2,c0)+(r2,c1) via the col-shifted copy
                at +SHIFT and closes the accumulation group.
                """
                co = conv_idx * 1152
                ps3 = ps.rearrange("p (h w) -> p h w", w=RW)
                pout = ps3[:, 0:nrows, 0:W]
                if step < 3:
                    c = step
                    rhs = bass.AP(
                        tensor=src.tensor,
                        offset=src.offset + h0 * RW + c,
                        ap=[src.ap[0], [RW, 2], [RW, nrows], [1, W]],
                    )
                    lhsT = w_sb[:, co + c * 256 : co + (c + 1) * 256].rearrange(
                        "p (j m) -> p j m", j=2
                    )
                    nc.tensor.matmul(
                        pout, lhsT, rhs, start=(c == 0), stop=False,
                        perf_mode=DR, skip_group_check=True,
                    )
                elif step == 3:
                    rhs = bass.AP(
                        tensor=src.tensor,
                        offset=src.offset + (h0 + 2) * RW + 2,
                        ap=[src.ap[0], [RW, nrows], [1, W]],
                    )
                    nc.tensor.matmul(
                        pout, w_sb[:, co + 1024 : co + 1152],
                        rhs, start=False, stop=False, skip_group_check=True,
                    )
                else:
                    rhs = bass.AP(
                        tensor=src.tensor,
                        offset=src.offset + (h0 + 2) * RW,
                        ap=[src.ap[0], [SHIFT, 2], [RW, nrows], [1, W]],
                    )
                    lhsT = w_sb[:, co + 768 : co + 1024].rearrange(
                        "p (j m) -> p j m", j=2
                    )
                    nc.tensor.matmul(
                        pout, lhsT, rhs, start=False, stop=True,
                        perf_mode=DR, skip_group_check=True,
                    )

            def conv_chunk(ps, src, conv_idx, h0, nrows=CHUNK_ROWS):
                for step in range(5):
                    chunk_matmul(ps, src, conv_idx, h0, step, nrows)

            def conv_chunk_pair(psA, psB, src, conv_idx, h0A, h0B):
                """Two chunks with interleaved matmuls: alternating the two
                PSUM accumulation groups hides the per-group start/stop
                transition bubble on the PE."""
                for step in range(5):
                    chunk_matmul(psA, src, conv_idx, h0A, step)
                    chunk_matmul(psB, src, conv_idx, h0B, step)

            # PE p-state warmup: the tensor clock ramps 0.65->2.4 GHz over
            # ~3us of continuous work, so burn the ramp on dummy matmuls
            # during the initial DMA window instead of on the real stream
            ps_warm = pspool.tile([C, NFLAT], F32, tag="ps1")
            lhsT_w = junk2[:, 0:256].rearrange("p (j m) -> p j m", j=2)
            for r in range(16):
                rhs = bass.AP(
                    tensor=junk2.tensor, offset=junk2.offset,
                    ap=[junk2.ap[0], [64, 2], [1, 224]],
                )
                nc.tensor.matmul(
                    ps_warm[:, 0:224], lhsT_w, rhs,
                    start=(r == 0), stop=(r == 15),
                    perf_mode=DR, skip_group_check=True,
                )

            # background one-time pad zeroing for the ts ring buffers (the
            # xs ring needs none: its pads arrive pre-baked from HBM)
            zero_pads(nc.vector, ts_t[0])
            zero_pads(nc.gpsimd, ts_t[1])
            zero_pads(nc.gpsimd, ts_t[2])

            for i in range(BL):
                xs, ts = xs_t[i % 3], ts_t[i % 3]
                xr, o = xr_t[i % 4], o_t[i % 2]
                ts3 = xs3v(ts)
                o3 = o.rearrange("p h w -> p h w")

                # conv1 input: the host-binarized padded sign image and its
                # col-shifted twin, both straight from HBM in row pieces
                for r0, nr in PIECES:
                    if not (i == 0 and r0 in (0, 11)):
                        a, b = r0 * RW, (r0 + nr) * RW
                        nc.sync.dma_start(xs[:, a:b], s_d[i, :, a:b])
                        nc.gpsimd.dma_start(
                            xs[:, SHIFT + a : SHIFT + b],
                            s_d[i, :, SHIFT + a : SHIFT + b],
                        )
                nc.gpsimd.dma_start(xr[:, 0:28, :], xr_d[i, :, 0:28, :])
                nc.gpsimd.dma_start(xr[:, 28:56, :], xr_d[i, :, 28:56, :])

                def evict1(ps1, h0):
                    # bn1 + sign (hardtanh folded into sign) -> conv2 input
                    ps1v = ps1.rearrange("p (h w) -> p h w", w=RW)[:, :, 0:W]
                    nc.scalar.activation(
                        ts3[:, 1 + h0 : 1 + h0 + CHUNK_ROWS, 1 : W + 1],
                        ps1v,
                        SIGN,
                        bias=bn_sb[:, 1:2],
                        scale=bn_sb[:, 0:1],
                    )
                    shift_dma(nc.gpsimd, ts, 1 + h0, CHUNK_ROWS)

                for k in range(0, N_CHUNKS - 1, 2):
                    h0A, h0B = k * CHUNK_ROWS, (k + 1) * CHUNK_ROWS
                    psA = pspool.tile([C, NFLAT], F32, tag="ps1")
                    psB = pspool.tile([C, NFLAT], F32, tag="ps1")
                    conv_chunk_pair(psA, psB, xs, 0, h0A, h0B)
                    evict1(psA, h0A)
                    evict1(psB, h0B)
                h0 = (N_CHUNKS - 1) * CHUNK_ROWS
                ps1 = pspool.tile([C, NFLAT], F32, tag="ps1")
                conv_chunk(ps1, xs, 0, h0)
                evict1(ps1, h0)

                def evict2(ps2, k):
                    h0 = k * CHUNK_ROWS
                    ps2v = ps2.rearrange("p (h w) -> p h w", w=RW)[:, :, 0:W]
                    # out = clip(ps2*inv2 + (x+b2), -1, 1). Odd chunks stage
                    # the PSUM affine through the lightly-loaded Scalar
                    # engine so DVE only does the cheaper add + clip.
                    ov = o3[:, h0 : h0 + CHUNK_ROWS, :]
                    if k % 2 == 1:
                        tv = tmp_t[(k // 2) % 2]
                        nc.scalar.activation(
                            tv[:], ps2v,
                            mybir.ActivationFunctionType.Identity,
                            scale=bn_sb[:, 2:3],
                        )
                        nc.vector.tensor_add(
                            ov, tv[:], xr[:, h0 : h0 + CHUNK_ROWS, :]
                        )
                    else:
                        nc.vector.affine_then_add(
                            ov, ps2v, xr[:, h0 : h0 + CHUNK_ROWS, :],
                            scale=bn_sb[:, 2:3], bias=0.0,
                        )
                    nc.vector.tensor_scalar(
                        ov, ov, 1.0, -1.0,
                        op0=mybir.AluOpType.min, op1=mybir.AluOpType.max,
                    )
                    # y issues ride the scalar queue: an in-order dma_start
                    # waits on the eviction, and on sync it would block the
                    # next image's input pieces behind it
                    if k == 3:
                        nc.scalar.dma_start(y_d[i, :, 0:32, :], o3[:, 0:32, :])
                    elif k == 6:
                        nc.scalar.dma_start(y_d[i, :, 32:56, :], o3[:, 32:56, :])
                    elif k == 5 and i == BL - 1:
                        nc.scalar.dma_start(y_d[i, :, 32:48, :], o3[:, 32:48, :])

                if i < BL - 1:
                    for k in range(0, N_CHUNKS - 1, 2):
                        psA = pspool.tile([C, NFLAT], F32, tag="ps2")
                        psB = pspool.tile([C, NFLAT], F32, tag="ps2")
                        conv_chunk_pair(
                            psA, psB, ts, 1, k * CHUNK_ROWS, (k + 1) * CHUNK_ROWS
                        )
                        evict2(psA, k)
                        evict2(psB, k + 1)
                    ps2 = pspool.tile([C, NFLAT], F32, tag="ps2")
                    conv_chunk(ps2, ts, 1, (N_CHUNKS - 1) * CHUNK_ROWS)
                    evict2(ps2, N_CHUNKS - 1)
                else:
                    # last image: sequential chunks alternating both psum
                    # tags (conv1 is finished, so its banks are free and no
                    # group ever waits on its own eviction). The eviction
                    # work alternates between the DVE path and a Scalar
                    # (Identity-act affine) + Pool (add/clip) path so no
                    # single engine backlogs behind the final matmuls, and
                    # the final chunk runs as two parallel 4-row groups.
                    IDENT = mybir.ActivationFunctionType.Identity

                    def evict2_sp(ps2, a, rows):
                        # Scalar does the PSUM affine; DVE only the (cheaper)
                        # add + clip, so neither engine backlogs at the tail
                        ps2v = ps2.rearrange("p (h w) -> p h w", w=RW)[
                            :, 0:rows, 0:W
                        ]
                        tv = tmp3[:, 0:rows, :]
                        nc.scalar.activation(
                            tv, ps2v, IDENT, scale=bn_sb[:, 2:3]
                        )
                        ov = o3[:, a : a + rows, :]
                        nc.vector.tensor_add(
                            ov, tv, xr[:, a : a + rows, :]
                        )
                        nc.vector.tensor_scalar(
                            ov, ov, 1.0, -1.0,
                            op0=mybir.AluOpType.min, op1=mybir.AluOpType.max,
                        )

                    def evict2_dve(ps2, a, rows):
                        ps2v = ps2.rearrange("p (h w) -> p h w", w=RW)[
                            :, 0:rows, 0:W
                        ]
                        ov = o3[:, a : a + rows, :]
                        nc.vector.affine_then_add(
                            ov, ps2v, xr[:, a : a + rows, :],
                            scale=bn_sb[:, 2:3], bias=0.0,
                        )
                        nc.vector.tensor_scalar(
                            ov, ov, 1.0, -1.0,
                            op0=mybir.AluOpType.min, op1=mybir.AluOpType.max,
                        )

                    for k in range(N_CHUNKS - 1):
                        tmp3 = tmp_t[(k // 2) % 2]
                        ps2 = pspool.tile(
                            [C, NFLAT], F32, tag=("ps1", "ps2")[k % 2]
                        )
                        conv_chunk(ps2, ts, 1, k * CHUNK_ROWS)
                        (evict2_dve if k % 2 == 0 or not SPLIT_EVICT
                         else evict2_sp)(ps2, k * CHUNK_ROWS, CHUNK_ROWS)
                        if k == 3:
                            nc.scalar.dma_start(
                                y_d[i, :, 0:32, :], o3[:, 0:32, :]
                            )
                        elif k == 5:
                            nc.scalar.dma_start(
                                y_d[i, :, 32:48, :], o3[:, 32:48, :]
                            )
                    h0 = (N_CHUNKS - 1) * CHUNK_ROWS
                    for half, evict, eng in (
                        (0, evict2_dve, nc.scalar),
                        (4, evict2_dve, nc.sync),
                    ):
                        tmp3 = tmp_t[0]
                        ps2 = pspool.tile(
                            [C, NFLAT], F32, tag=("ps1", "ps2")[half % 3]
                        )
                        conv_chunk(ps2, ts, 1, h0 + half, nrows=4)
                        evict(ps2, h0 + half, 4)
                        eng.dma_start(
                            y_d[i, :, h0 + half : h0 + half + 4, :],
                            o3[:, h0 + half : h0 + half + 4, :],
                        )

    nc.compile()
    return nc


def _get_nc():
    global _NC_CACHE
    if _NC_CACHE is None:
        _NC_CACHE = _build_nc()
    return _NC_CACHE


def kernel(
    x, w1, w2, gamma1, beta1, mean1, var1, gamma2, beta2, mean2, var2,
    trace=False,
):
    x = np.asarray(x, dtype=np.float32)
    w1 = np.asarray(w1, dtype=np.float32)
    w2 = np.asarray(w2, dtype=np.float32)

    # fold BN exactly as the reference does (f32 throughout)
    def fold(gamma, beta, mean, var):
        inv = (np.asarray(gamma, np.float32)
               / np.sqrt(np.asarray(var, np.float32) + np.float32(EPS)))
        b = np.asarray(beta, np.float32) - np.asarray(mean, np.float32) * inv
        return inv.astype(np.float32), b.astype(np.float32)

    inv1, b1 = fold(gamma1, beta1, mean1, var1)
    inv2, b2 = fold(gamma2, beta2, mean2, var2)
    bn_np = np.stack([inv1, b1, inv2, b2], axis=1).astype(np.float32)  # [C,4]

    # host prep: binarized input in the padded row-stride-64 layout with its
    # col-shifted twin appended, plus the b2-biased residual
    sg = np.sign(x).astype(ml_dtypes.float8_e4m3fn)
    sp = np.zeros((B, C, 2 * HP, RW), dtype=ml_dtypes.float8_e4m3fn)
    sp[:, :, 1 : H + 1, 1 : W + 1] = sg
    sp[:, :, HP + 1 : HP + H + 1, 0:W] = sg
    s_np = sp.reshape(B, C, 2 * SHIFT)
    xr_np = (x + b2[None, :, None, None]).astype(ml_dtypes.bfloat16)

    # fp8 weight tables; per conv: 3 DoubleRow pair tables, the (r2,c0)+
    # (r2,c1) pair, then the plain (r2,c2) table.
    # DR c=0..2: w_np[k, co + c*256 + j*128 + m] = sign(w[m,k,j,c]), j=row 0/1
    # DR #4:     pairs (r2,c0) j=0 and (r2,c1) j=1 at co+768
    # normal:    (r2,c2) at co+1024
    w_np = np.empty((C, WCOLS), dtype=ml_dtypes.float8_e4m3fn)
    for conv_idx, w in enumerate((w1, w2)):
        ws = np.sign(w).astype(ml_dtypes.float8_e4m3fn)  # [O, Cin, 3, 3]
        co = conv_idx * 1152
        for c in range(3):
            for j in range(2):
                w_np[:, co + c * 256 + j * 128 : co + c * 256 + (j + 1) * 128] = (
                    ws[:, :, j, c].T
                )
        w_np[:, co + 768 : co + 896] = ws[:, :, 2, 0].T
        w_np[:, co + 896 : co + 1024] = ws[:, :, 2, 1].T
        w_np[:, co + 1024 : co + 1152] = ws[:, :, 2, 2].T

    nc = _get_nc()
    in_maps = [
        {
            "s": s_np[i * BL : (i + 1) * BL],
            "xr": xr_np[i * BL : (i + 1) * BL],
            "w": w_np,
            "bn": bn_np,
        }
        for i in range(N_CORES)
    ]
    res = run_bass_kernel_spmd(
        nc, in_maps, core_ids=list(range(N_CORES)), trace=trace
    )
    y = np.concatenate(
        [np.asarray(res.results[i]["y"]) for i in range(N_CORES)], axis=0
    ).astype(np.float32)
    if trace:
        return y, res
    return y


# revision 66
# speedup vs baseline: 1.0285x; 1.0285x over previous
"""Trainium2 Bass kernel for nn_BasicBlock (binarized ResNet basic block).

Computation (see problem reference):
    residual = x
    out = psum_conv3x3(sign(x), sign(w1))        # 3x3 'same' conv, saturating acc
    out = bn1(out); out = hardtanh(out)
    out = psum_conv3x3(sign(out), sign(w2))
    out = bn2(out); out = out + residual; out = hardtanh(out)

Key facts exploited:
  * C=128 channels = one GROUP, 9 taps of |partial| <= 128 each, so the
    running accumulator magnitude is <= 9*128 = 1152 < THRESH=8000: the
    saturation clip NEVER binds. The conv is a plain 3x3 conv over sign
    values, all arithmetic exact small integers -> freely reorderable and
    exactly representable in fp8e4/bf16 inputs with fp32 PSUM accumulation.
  * sign(hardtanh(v)) == sign(v), so the first hardtanh can be folded into
    the sign feeding conv2.
  * Each conv = 9 shifted-window taps (K=C=128 on partitions) into one PSUM
    accumulation group over a zero-padded row-stride-64 fp8 sign image:
    4 fp8 DoubleRow matmuls (vertically adjacent tap pairs at +RW, plus the
    (r2,c0)+(r2,c1) pair via a col-shifted copy at +SHIFT) and 1 normal
    fp8 matmul for the odd tap (r2,c2).
  * Host-side prep: sign(x) is computed on the host and shipped as fp8
    (both the padded image and its col-shifted twin are plain DMAs from the
    same HBM tensor), and the residual arrives as x+b2 in bf16 (bias of the
    second BN folded in; sign() and +b2 are exact/cheap host prep like the
    weight binarization). The second conv's input sign image is produced
    on-chip by the Scalar engine (bn1+sign straight out of PSUM); its
    shifted twin is an SBUF->SBUF DMA on the otherwise idle DMA rings.
  * y is returned as bf16 (quantization ~2^-9 against a 2e-2 budget).

Sharding: data-parallel over batch: 64 images -> 8 cores x 8 images.
"""

import sys

sys.path.insert(0, "/opt/trn_rl_repo")

import numpy as np
import ml_dtypes

import concourse.bass as bass
import concourse.bacc as bacc
import concourse.mybir as mybir
import concourse.tile as tile
from concourse.bass_utils import run_bass_kernel_spmd

# ---------------------------------------------------------------- constants

N_CORES = 8
B, C, H, W = 64, 128, 56, 56
BL = B // N_CORES            # images per core
HP = H + 2                   # padded rows
RW = 64                      # padded row width (stride): 56 valid + pads,
                             # 64 so the DoubleRow plane step (+RW) is 16-aligned
CHUNK_ROWS = 8               # output rows per PSUM chunk
NFLAT = CHUNK_ROWS * RW      # 512 flat psum columns per chunk (one bank)
N_CHUNKS = H // CHUNK_ROWS   # 7
EPS = 1e-5
SHIFT = HP * RW              # offset of the col-shifted copy inside xs/ts
WCOLS = 2 * (4 * 256 + 128)  # fp8 weight table columns (2 convs x 1152)
PIECES = ((0, 11), (11, 24), (35, 23))  # sign-image DMA pieces (padded rows)

F32 = mybir.dt.float32
BF16 = mybir.dt.bfloat16
FP8 = mybir.dt.float8e4

# perf_mode for the odd (r2,c2) tap. DoublePixel compiles and produces
# correct results but is silently dropped before NEFF encoding (no perf_opt
# bit in the emitted instruction), so it buys nothing; keep None.
DP_SINGLE = None
# last image: alternate evictions between the DVE path and Scalar+DVE path
SPLIT_EVICT = False

_NC_CACHE = None


def _build_nc():
    """Build the per-core Bass module (same NEFF on all 8 cores)."""
    nc = bacc.Bacc("TRN2", debug=False)

    # host-binarized sign(x) in fp8, already laid out as the zero-padded
    # row-stride-64 image followed by its col-shifted twin (so every piece
    # DMA is contiguous on both sides), and the b2-biased residual in bf16
    s_d = nc.dram_tensor("s", [BL, C, 2 * SHIFT], FP8, kind="ExternalInput").ap()
    xr_d = nc.dram_tensor("xr", [BL, C, H, W], BF16, kind="ExternalInput").ap()
    # host-prepped fp8 weight tables, per conv: 3 DoubleRow pair tables
    # [cin, 2*cout] for (r0,r1) at c=0,1,2 then the (r2,c0)+(r2,c1) pair and
    # the plain (r2,c2) table
    w_d = nc.dram_tensor("w", [C, WCOLS], FP8, kind="ExternalInput").ap()
    # folded BN params per channel: [:,0]=inv1 [:,1]=b1 [:,2]=inv2
    bn_d = nc.dram_tensor("bn", [C, 4], F32, kind="ExternalInput").ap()
    y_d = nc.dram_tensor("y", [BL, C, H, W], BF16, kind="ExternalOutput").ap()

    SIGN = mybir.ActivationFunctionType.Sign
    DR = mybir.MatmulPerfMode.DoubleRow

    with tile.TileContext(nc) as tc:
        with (
            tc.tile_pool(name="sb", bufs=1) as sb,
            tc.tile_pool(name="psum", bufs=4, space="PSUM") as pspool,
        ):
            # -- startup: preload the SIGN activation table while DMAs run
            junk = sb.tile([C, 2], F32, name="junk")
            nc.vector.memset(junk[:], 0.0)
            nc.scalar.activation(junk[:, 1:2], junk[:, 0:1], SIGN)
            # fp8 scratch feeding the PE-clock warmup matmuls
            junk2 = sb.tile([C, 512], FP8, name="junk2")
            nc.vector.memset(junk2[:], 0.0)

            w_sb = sb.tile([C, WCOLS], FP8, name="wsb")
            bn_sb = sb.tile([C, 4], F32, name="bnsb")
            xs_t = [sb.tile([C, 2 * SHIFT], FP8, name=f"xs{j}") for j in range(3)]
            ts_t = [sb.tile([C, 2 * SHIFT], FP8, name=f"ts{j}") for j in range(3)]
            # ring depth 4: the write-after-read dependency of image i's
            # residual load then lands on image i-4's (long finished) final
            # eviction, so the issue never blocks its queue
            xr_t = [sb.tile([C, H, W], BF16, name=f"xr{j}") for j in range(4)]
            o_t = [sb.tile([C, H, W], BF16, name=f"o{j}") for j in range(2)]
            # f32 staging for the last image's Scalar+Pool eviction path
            tmp_t = [
                sb.tile([C, CHUNK_ROWS, W], F32, name=f"tmp{j}")
                for j in range(2)
            ]

            def xs3v(buf):
                return buf[:, 0:SHIFT].rearrange("p (h w) -> p h w", w=RW)

            def xsh3v(buf):
                return buf[:, SHIFT : 2 * SHIFT].rearrange(
                    "p (h w) -> p h w", w=RW
                )

            def zero_pads(eng, buf):
                b3 = xs3v(buf)
                eng.memset(b3[:, 0, :], 0.0)
                eng.memset(b3[:, HP - 1, :], 0.0)
                eng.memset(b3[:, 1 : HP - 1, 0:1], 0.0)
                eng.memset(b3[:, 1 : HP - 1, W + 1 : RW], 0.0)
                # last padded row of the shifted copy is all pad-derived
                eng.memset(buf[:, SHIFT + (HP - 1) * RW : 2 * SHIFT], 0.0)

            # ramp-ordered queues. Sync carries only the main sign-image
            # pieces (plus conv1 weights); gpsimd carries the shifted twins,
            # bn, conv2 weights and residuals — neither queue ever backs up
            # behind the other's latency-critical issues.
            nc.sync.dma_start(
                xs_t[0][:, 0 : 11 * RW], s_d[0, :, 0 : 11 * RW]
            )
            nc.sync.dma_start(w_sb[:, 0:1152], w_d[:, 0:1152])
            nc.sync.dma_start(
                xs_t[0][:, 11 * RW : 35 * RW], s_d[0, :, 11 * RW : 35 * RW]
            )
            nc.gpsimd.dma_start(
                xs_t[0][:, SHIFT : SHIFT + 11 * RW],
                s_d[0, :, SHIFT : SHIFT + 11 * RW],
            )
            nc.gpsimd.dma_start(bn_sb[:], bn_d[:])
            nc.gpsimd.dma_start(
                xs_t[0][:, SHIFT + 11 * RW : SHIFT + 35 * RW],
                s_d[0, :, SHIFT + 11 * RW : SHIFT + 35 * RW],
            )
            nc.gpsimd.dma_start(w_sb[:, 1152:WCOLS], w_d[:, 1152:WCOLS])

            def shift_dma(eng, buf, row0, nrows):
                """shifted[h, w] = main[h, w+1] for rows [row0, row0+nrows)
                via SBUF->SBUF DMA (pad cols supply the tail bytes)."""
                src = bass.AP(
                    tensor=buf.tensor,
                    offset=buf.offset + row0 * RW + 1,
                    ap=[buf.ap[0], [1, nrows * RW]],
                )
                dst = bass.AP(
                    tensor=buf.tensor,
                    offset=buf.offset + SHIFT + row0 * RW,
                    ap=[buf.ap[0], [1, nrows * RW]],
                )
                eng.dma_start(dst, src)

            def chunk_matmul(ps, src, conv_idx, h0, step, nrows=CHUNK_ROWS):
                """Emit matmul `step` (0..4) of one output chunk: 4 DoubleRow
                + 1 normal fp8 matmul.

                Steps 0..2 pair the vertically adjacent taps (r0,c)+(r1,c)
                (planes at +RW). Step 3 is the odd tap (r2,c2) as a normal
                matmul (before step 4 so a late shifted copy never stalls
                it); step 4 pairs (r2,c0)+(r2,c1) via the col-shifted copy
                at +SHIFT and closes the accumulation group.
                """
                co = conv_idx * 1152
                ps3 = ps.rearrange("p (h w) -> p h w", w=RW)
                pout = ps3[:, 0:nrows, 0:W]
                if step < 3:
                    c = step
                    rhs = bass.AP(
                        tensor=src.tensor,
                        offset=src.offset + h0 * RW + c,
                        ap=[src.ap[0], [RW, 2], [RW, nrows], [1, W]],
                    )
                    lhsT = w_sb[:, co + c * 256 : co + (c + 1) * 256].rearrange(
                        "p (j m) -> p j m", j=2
                    )
                    nc.tensor.matmul(
                        pout, lhsT, rhs, start=(c == 0), stop=False,
                        perf_mode=DR, skip_group_check=True,
                    )
                elif step == 3:
                    rhs = bass.AP(
                        tensor=src.tensor,
                        offset=src.offset + (h0 + 2) * RW + 2,
                        ap=[src.ap[0], [RW, nrows], [1, W]],
                    )
                    nc.tensor.matmul(
                        pout, w_sb[:, co + 1024 : co + 1152],
                        rhs, start=False, stop=False, skip_group_check=True,
                    )
                else:
                    rhs = bass.AP(
                        tensor=src.tensor,
                        offset=src.offset + (h0 + 2) * RW,
                        ap=[src.ap[0], [SHIFT, 2], [RW, nrows], [1, W]],
                    )
                    lhsT = w_sb[:, co + 768 : co + 1024].rearrange(
                        "p (j m) -> p j m", j=2
                    )
                    nc.tensor.matmul(
                        pout, lhsT, rhs, start=False, stop=True,
                        perf_mode=DR, skip_group_check=True,
                    )

            def conv_chunk(ps, src, conv_idx, h0, nrows=CHUNK_ROWS):
                for step in range(5):
                    chunk_matmul(ps, src, conv_idx, h0, step, nrows)

            def conv_chunk_pair(psA, psB, src, conv_idx, h0A, h0B):
                """Two chunks with interleaved matmuls: alternating the two
                PSUM accumulation groups hides the per-group start/stop
                transition bubble on the PE."""
                for step in range(5):
                    chunk_matmul(psA, src, conv_idx, h0A, step)
                    chunk_matmul(psB, src, conv_idx, h0B, step)

            # PE p-state warmup: the tensor clock ramps 0.65->2.4 GHz over
            # ~3us of continuous work, so burn the ramp on dummy matmuls
            # during the initial DMA window instead of on the real stream
            ps_warm = pspool.tile([C, NFLAT], F32, tag="ps1")
            lhsT_w = junk2[:, 0:256].rearrange("p (j m) -> p j m", j=2)
            for r in range(16):
                rhs = bass.AP(
                    tensor=junk2.tensor, offset=junk2.offset,
                    ap=[junk2.ap[0], [64, 2], [1, 224]],
                )
                nc.tensor.matmul(
                    ps_warm[:, 0:224], lhsT_w, rhs,
                    start=(r == 0), stop=(r == 15),
                    perf_mode=DR, skip_group_check=True,
                )

            # background one-time pad zeroing for the ts ring buffers (the
            # xs ring needs none: its pads arrive pre-baked from HBM)
            zero_pads(nc.vector, ts_t[0])
            zero_pads(nc.gpsimd, ts_t[1])
            zero_pads(nc.gpsimd, ts_t[2])

            for i in range(BL):
                xs, ts = xs_t[i % 3], ts_t[i % 3]
                xr, o = xr_t[i % 4], o_t[i % 2]
                ts3 = xs3v(ts)
                o3 = o.rearrange("p h w -> p h w")

                # conv1 input: the host-binarized padded sign image and its
                # col-shifted twin, both straight from HBM in row pieces
                for r0, nr in PIECES:
                    if not (i == 0 and r0 in (0, 11)):
                        a, b = r0 * RW, (r0 + nr) * RW
                        nc.sync.dma_start(xs[:, a:b], s_d[i, :, a:b])
                        nc.gpsimd.dma_start(
                            xs[:, SHIFT + a : SHIFT + b],
                            s_d[i, :, SHIFT + a : SHIFT + b],
                        )
                nc.gpsimd.dma_start(xr[:, 0:28, :], xr_d[i, :, 0:28, :])
                nc.gpsimd.dma_start(xr[:, 28:56, :], xr_d[i, :, 28:56, :])

                def evict1(ps1, h0):
                    # bn1 + sign (hardtanh folded into sign) -> conv2 input
                    ps1v = ps1.rearrange("p (h w) -> p h w", w=RW)[:, :, 0:W]
                    nc.scalar.activation(
                        ts3[:, 1 + h0 : 1 + h0 + CHUNK_ROWS, 1 : W + 1],
                        ps1v,
                        SIGN,
                        bias=bn_sb[:, 1:2],
                        scale=bn_sb[:, 0:1],
                    )
                    shift_dma(nc.gpsimd, ts, 1 + h0, CHUNK_ROWS)

                for k in range(0, N_CHUNKS - 1, 2):
                    h0A, h0B = k * CHUNK_ROWS, (k + 1) * CHUNK_ROWS
                    psA = pspool.tile([C, NFLAT], F32, tag="ps1")
                    psB = pspool.tile([C, NFLAT], F32, tag="ps1")
                    conv_chunk_pair(psA, psB, xs, 0, h0A, h0B)
                    evict1(psA, h0A)
                    evict1(psB, h0B)
                h0 = (N_CHUNKS - 1) * CHUNK_ROWS
                ps1 = pspool.tile([C, NFLAT], F32, tag="ps1")
                conv_chunk(ps1, xs, 0, h0)
                evict1(ps1, h0)

                def evict2(ps2, k):
                    h0 = k * CHUNK_ROWS
                    ps2v = ps2.rearrange("p (h w) -> p h w", w=RW)[:, :, 0:W]
                    # out = clip(ps2*inv2 + (x+b2), -1, 1): one fused DVE op
                    # + one min/max clip, written straight to the bf16 output
                    ov = o3[:, h0 : h0 + CHUNK_ROWS, :]
                    nc.vector.affine_then_add(
                        ov, ps2v, xr[:, h0 : h0 + CHUNK_ROWS, :],
                        scale=bn_sb[:, 2:3], bias=0.0,
                    )
                    nc.vector.tensor_scalar(
                        ov, ov, 1.0, -1.0,
                        op0=mybir.AluOpType.min, op1=mybir.AluOpType.max,
                    )
                    # y issues ride the scalar queue: an in-order dma_start
                    # waits on the eviction, and on sync it would block the
                    # next image's input pieces behind it
                    if k == 3:
                        nc.scalar.dma_start(y_d[i, :, 0:32, :], o3[:, 0:32, :])
                    elif k == 6:
                        nc.scalar.dma_start(y_d[i, :, 32:56, :], o3[:, 32:56, :])
                    elif k == 5 and i == BL - 1:
                        nc.scalar.dma_start(y_d[i, :, 32:48, :], o3[:, 32:48, :])

                if i < BL - 1:
                    for k in range(0, N_CHUNKS - 1, 2):
                        psA = pspool.tile([C, NFLAT], F32, tag="ps2")
                        psB = pspool.tile([C, NFLAT], F32, tag="ps2")
                        conv_chunk_pair(
                            psA, psB, ts, 1, k * CHUNK_ROWS, (k + 1) * CHUNK_ROWS
                        )
                        evict2(psA, k)
                        evict2(psB, k + 1)
                    ps2 = pspool.tile([C, NFLAT], F32, tag="ps2")
                    conv_chunk(ps2, ts, 1, (N_CHUNKS - 1) * CHUNK_ROWS)
                    evict2(ps2, N_CHUNKS - 1)
                else:
                    # last image: sequential chunks alternating both psum
                    # tags (conv1 is finished, so its banks are free and no
                    # group ever waits on its own eviction). The eviction
                    # work alternates between the DVE path and a Scalar
                    # (Identity-act affine) + Pool (add/clip) path so no
                    # single engine backlogs behind the final matmuls, and
                    # the final chunk runs as two parallel 4-row groups.
                    IDENT = mybir.ActivationFunctionType.Identity

                    def evict2_sp(ps2, a, rows):
                        # Scalar does the PSUM affine; DVE only the (cheaper)
                        # add + clip, so neither engine backlogs at the tail
                        ps2v = ps2.rearrange("p (h w) -> p h w", w=RW)[
                            :, 0:rows, 0:W
                        ]
                        tv = tmp3[:, 0:rows, :]
                        nc.scalar.activation(
                            tv, ps2v, IDENT, scale=bn_sb[:, 2:3]
                        )
                        ov = o3[:, a : a + rows, :]
                        nc.vector.tensor_add(
                            ov, tv, xr[:, a : a + rows, :]
                        )
                        nc.vector.tensor_scalar(
                            ov, ov, 1.0, -1.0,
                            op0=mybir.AluOpType.min, op1=mybir.AluOpType.max,
                        )

                    def evict2_dve(ps2, a, rows):
                        ps2v = ps2.rearrange("p (h w) -> p h w", w=RW)[
                            :, 0:rows, 0:W
                        ]
                        ov = o3[:, a : a + rows, :]
                        nc.vector.affine_then_add(
                            ov, ps2v, xr[:, a : a + rows, :],
                            scale=bn_sb[:, 2:3], bias=0.0,
                        )
                        nc.vector.tensor_scalar(
                            ov, ov, 1.0, -1.0,
                            op0=mybir.AluOpType.min, op1=mybir.AluOpType.max,
                        )

                    for k in range(N_CHUNKS - 1):
                        tmp3 = tmp_t[(k // 2) % 2]
                        ps2 = pspool.tile(
                            [C, NFLAT], F32, tag=("ps1", "ps2")[k % 2]
                        )
                        conv_chunk(ps2, ts, 1, k * CHUNK_ROWS)
                        (evict2_dve if k % 2 == 0 or not SPLIT_EVICT
                         else evict2_sp)(ps2, k * CHUNK_ROWS, CHUNK_ROWS)
                        if k == 3:
                            nc.scalar.dma_start(
                                y_d[i, :, 0:32, :], o3[:, 0:32, :]
                            )
                        elif k == 5:
                            nc.scalar.dma_start(
                                y_d[i, :, 32:48, :], o3[:, 32:48, :]
                            )
                    h0 = (N_CHUNKS - 1) * CHUNK_ROWS
                    for half, evict, eng in (
                        (0, evict2_dve, nc.scalar),
                        (4, evict2_dve, nc.sync),
                    ):
                        tmp3 = tmp_t[0]
                        ps2 = pspool.tile(
                            [C, NFLAT], F32, tag=("ps1", "ps2")[half % 3]
                        )
                        conv_chunk(ps2, ts, 1, h0 + half, nrows=4)
                        evict(ps2, h0 + half, 4)
                        eng.dma_start(
                            y_d[i, :, h0 + half : h0 + half + 4, :],
                            o3[:, h0 + half : h0 + half + 4, :],
                        )

    nc.compile()
    return nc


def _get_nc():
    global _NC_CACHE
    if _NC_CACHE is None:
        _NC_CACHE = _build_nc()
    return _NC_CACHE


def kernel(
    x, w1, w2, gamma1, beta1, mean1, var1, gamma2, beta2, mean2, var2,
    trace=False,
):
    x = np.asarray(x, dtype=np.float32)
    w1 = np.asarray(w1, dtype=np.float32)
    w2 = np.asarray(w2, dtype=np.float32)

    # fold BN exactly as the reference does (f32 throughout)
    def fold(gamma, beta, mean, var):
        inv = (np.asarray(gamma, np.float32)
               / np.sqrt(np.asarray(var, np.float32) + np.float32(EPS)))
        b = np.asarray(beta, np.float32) - np.asarray(mean, np.float32) * inv
        return inv.astype(np.float32), b.astype(np.float32)

    inv1, b1 = fold(gamma1, beta1, mean1, var1)
    inv2, b2 = fold(gamma2, beta2, mean2, var2)
    bn_np = np.stack([inv1, b1, inv2, b2], axis=1).astype(np.float32)  # [C,4]

    # host prep: binarized input in the padded row-stride-64 layout with its
    # col-shifted twin appended, plus the b2-biased residual
    sg = np.sign(x).astype(ml_dtypes.float8_e4m3fn)
    sp = np.zeros((B, C, 2 * HP, RW), dtype=ml_dtypes.float8_e4m3fn)
    sp[:, :, 1 : H + 1, 1 : W + 1] = sg
    sp[:, :, HP + 1 : HP + H + 1, 0:W] = sg
    s_np = sp.reshape(B, C, 2 * SHIFT)
    xr_np = (x + b2[None, :, None, None]).astype(ml_dtypes.bfloat16)

    # fp8 weight tables; per conv: 3 DoubleRow pair tables, the (r2,c0)+
    # (r2,c1) pair, then the plain (r2,c2) table.
    # DR c=0..2: w_np[k, co + c*256 + j*128 + m] = sign(w[m,k,j,c]), j=row 0/1
    # DR #4:     pairs (r2,c0) j=0 and (r2,c1) j=1 at co+768
    # normal:    (r2,c2) at co+1024
    w_np = np.empty((C, WCOLS), dtype=ml_dtypes.float8_e4m3fn)
    for conv_idx, w in enumerate((w1, w2)):
        ws = np.sign(w).astype(ml_dtypes.float8_e4m3fn)  # [O, Cin, 3, 3]
        co = conv_idx * 1152
        for c in range(3):
            for j in range(2):
                w_np[:, co + c * 256 + j * 128 : co + c * 256 + (j + 1) * 128] = (
                    ws[:, :, j, c].T
                )
        w_np[:, co + 768 : co + 896] = ws[:, :, 2, 0].T
        w_np[:, co + 896 : co + 1024] = ws[:, :, 2, 1].T
        w_np[:, co + 1024 : co + 1152] = ws[:, :, 2, 2].T

    nc = _get_nc()
    in_maps = [
        {
            "s": s_np[i * BL : (i + 1) * BL],
            "xr": xr_np[i * BL : (i + 1) * BL],
            "w": w_np,
            "bn": bn_np,
        }
        for i in range(N_CORES)
    ]
    res = run_bass_kernel_spmd(
        nc, in_maps, core_ids=list(range(N_CORES)), trace=trace
    )
    y = np.concatenate(
        [np.asarray(res.results[i]["y"]) for i in range(N_CORES)], axis=0
    ).astype(np.float32)
    if trace:
        return y, res
    return y
